# revision 22
# baseline (speedup 1.0000x reference)
"""BitNet FFN (bitlinear158 -> gelu -> bitlinear158) Trainium2 kernel.

Sharding: data-parallel over tokens across 8 cores (1024 tokens/core).
Layout: tokens on the free axis everywhere; weights stationary in the PE.

Math notes (exactness):
  - activation quant ints = round(x * 127 / max|x|)  (the rms-norm cancels)
  - weight quant ternary = clip(round(w / clip(mean|w|,1e-5)), -1, 1)
  - both exactly representable in bf16; PSUM accumulates integer products
    (<= 2^21) exactly in fp32, so the matmuls are exact.
  - per-token output scale alpha = clip(max|x|*sqrt(d)/||x||, 1e-5)
      * clip(mean|w|,1e-5) / 127 applied on PSUM before gelu.
  - round-to-nearest-even via fp32 (t + 1.5*2^23) - 1.5*2^23, matching
    jnp.round; clip(round(t),-1,1) == round(clamp(t, +-1.4999999)).
  - mean|w| needs the full tensor: each core reduces its row-shard, then a
    tiny AllReduce combines the partial sums.
"""

import sys

for _p in ("/opt/trn_rl_repo", "/opt/trn_rl_repo/concourse"):
    if _p not in sys.path:
        sys.path.insert(0, _p)

import numpy as np

import concourse.bass as bass
import concourse.bacc as bacc
import concourse.mybir as mybir
import concourse.tile as tile
from concourse import library_config
from concourse.bass import ts
from concourse.masks import make_identity

F32 = mybir.dt.float32
BF16 = mybir.dt.bfloat16
AX = mybir.AxisListType.X
OP = mybir.AluOpType
AF = mybir.ActivationFunctionType

C_ROUND = 12582912.0  # 1.5 * 2**23 : fp32 RNE rounding constant
W_CLIP = 1.4999999    # round(clamp(t, +-W_CLIP)) == clip(round(t), -1, 1)
N_CORES = 8


def build_bitnet(D, I, T, n_cores=N_CORES, gelu_mode="gelu"):
    """Per-core SPMD Bass program.

    Per-core I/O: xT [D,T] f32 (token shard, transposed), w1T [D,I] f32 and
    w2T [I,D] f32 (full transposed weights), w1s [D/n,I] / w2s [I/n,D]
    (this core's rows, for the mean|w| partial) -> outT [D,T] f32.
    """
    KD = D // 128   # d tiles (layer-1 contraction; layer-2 output rows)
    KI = I // 128   # inner tiles
    TH = T // 2     # matmul moving free dim
    TJ = T // 128   # token tiles for stats transposes
    JD2 = D // 256  # paired output-column strips in layer 2
    K2H = KI // 2   # half of inner tiles (layer-2 weight streaming)
    R1 = D // n_cores   # w1 shard rows per core
    R2 = I // n_cores   # w2 shard rows per core
    A1 = (R1 + 127) // 128
    A2 = (R2 + 127) // 128
    inv_cnt = 1.0 / float(D * I)
    sqrt_d = float(np.sqrt(np.float64(D)))
    sqrt_i = float(np.sqrt(np.float64(I)))

    nc = bacc.Bacc("TRN2", num_devices=n_cores)

    xT = nc.dram_tensor("xT", [D, T], F32, kind="ExternalInput")
    w1T = nc.dram_tensor("w1T", [D, I], F32, kind="ExternalInput")
    w2T = nc.dram_tensor("w2T", [I, D], F32, kind="ExternalInput")
    w1s = nc.dram_tensor("w1s", [D // n_cores, I], F32, kind="ExternalInput")
    w2s = nc.dram_tensor("w2s", [I // n_cores, D], F32, kind="ExternalInput")
    outT = nc.dram_tensor("outT", [D, T], F32, kind="ExternalOutput")

    h_dram = nc.dram_tensor("h_scratch", [I, T], F32, kind="Internal")
    w2q_dram = nc.dram_tensor("w2q_scratch", [I, D], BF16, kind="Internal")
    ar_in = nc.dram_tensor("ar_in", [8], F32, kind="Internal")
    ar_out = nc.dram_tensor("ar_out", [8], F32, kind="Internal",
                            addr_space="Shared")
    stat_dram = nc.dram_tensor("stat_dram", [6, T], F32, kind="Internal")
    srow_v = stat_dram.ap()                                     # [6, T]
    stok_v = stat_dram.ap().rearrange("r (j p) -> r p j", p=128)  # [6,128,TJ]

    xT_t = xT.ap().rearrange("(k p) t -> k p t", p=128)           # [KD,128,T]
    w1_t = w1T.ap().rearrange("(k p) (i j) -> i p k j", p=128, j=128)
    w1s_ap = w1s.ap()
    w2s_ap = w2s.ap()
    w2_t = w2T.ap().rearrange("(q p) d -> q p d", p=128)          # [KI,128,D]
    w2q_w = w2q_dram.ap().rearrange("(q p) d -> q p d", p=128)
    w2q_r = w2q_dram.ap().rearrange("(k p) (m c) -> m p k c", p=128, c=256)
    h_w = h_dram.ap().rearrange("(k p) t -> k p t", p=128)
    out_w = outT.ap().rearrange("(k p) t -> k p t", p=128)

    with tile.TileContext(nc) as tc:
        with (
            tc.tile_pool(name="glob", bufs=1) as glob,
            tc.tile_pool(name="psum", bufs=8, space="PSUM") as psum,
            tc.tile_pool(name="stats", bufs=1) as stats,
        ):
            # --- persistent constants & small tiles ---
            ones_bf = glob.tile([128, 1], BF16)
            nc.vector.memset(ones_bf, 1.0)
            ident = glob.tile([128, 128], F32)
            make_identity(nc, ident)
            wsc = glob.tile([128, 4], F32)   # cols: s1, s2, mclip1, mclip2
            qs1_b = glob.tile([128, T], F32, tag="qsb")
            al1_b = glob.tile([128, T], F32, tag="alb")

            # stats layout shuffles go through DRAM rows: token t = 128*j + p

            def part_reduce_max(acc, res):
                # reduce [128, T] over partitions -> res [128, TJ] tok-part
                for j in range(TJ):
                    trp = psum.tile([128, 128], F32, tag="b", name="trp")
                    nc.tensor.transpose(trp[:, :], acc[:, ts(j, 128)],
                                        ident[:, :])
                    nc.vector.tensor_reduce(
                        out=res[:, j:j + 1], in_=trp[:, :], axis=AX, op=OP.max)

            def finalize_stats(Mx, srow, mclip_col, sqrt_dim, qs_b, al_b, r0):
                """Mx [128,TJ] tok-part absmax; srow [1,T] sumsq row.
                Builds qs_b = 127/max|x| and al_b = per-token dequant scale,
                both broadcast to [128, T]. r0: base row in stat_dram."""
                nc.sync.dma_start(out=srow_v[r0:r0 + 1, :], in_=srow[:, :])
                ssq = stats.tile([128, TJ], F32, name="ssq")
                nc.sync.dma_start(out=ssq[:, :], in_=stok_v[r0])
                nrm = stats.tile([128, TJ], F32, name="nrm")
                nc.vector.tensor_scalar(nrm, ssq, 1e-38, None, OP.max)
                nc.scalar.activation(nrm, nrm, AF.Sqrt)
                nc.vector.tensor_scalar(nrm, nrm, 1e-12, None, OP.max)
                inv_n = stats.tile([128, TJ], F32, name="inv_n")
                nc.vector.reciprocal(inv_n, nrm)
                al = stats.tile([128, TJ], F32, name="al")
                nc.vector.tensor_tensor(al, Mx, inv_n, OP.mult)
                nc.vector.tensor_scalar(al, al, sqrt_dim, 1e-5, OP.mult, OP.max)
                nc.vector.tensor_scalar(al, al, wsc[:, mclip_col:mclip_col + 1],
                                        1.0 / 127.0, OP.mult, OP.mult)
                qs = stats.tile([128, TJ], F32, name="qs")
                nc.vector.tensor_scalar(qs, Mx, 1e-30, None, OP.max)
                nc.vector.reciprocal(qs, qs)
                nc.vector.tensor_scalar(qs, qs, 127.0, None, OP.mult)
                nc.sync.dma_start(out=stok_v[r0 + 1], in_=qs[:, :])
                nc.sync.dma_start(out=stok_v[r0 + 2], in_=al[:, :])
                qrow = stats.tile([1, T], F32, name="qrow")
                arow = stats.tile([1, T], F32, name="arow")
                nc.sync.dma_start(out=qrow[:, :], in_=srow_v[r0 + 1:r0 + 2, :])
                nc.sync.dma_start(out=arow[:, :], in_=srow_v[r0 + 2:r0 + 3, :])
                nc.gpsimd.partition_broadcast(qs_b[:, :], qrow[:, :])
                nc.gpsimd.partition_broadcast(al_b[:, :], arow[:, :])

            # ========= Stage A: weight scale partials + AllReduce =========
            with tc.tile_pool(name="wredp", bufs=2) as wredp:
                wps = stats.tile([128, A1 + A2], F32)
                if R1 % 128 or R2 % 128:  # partial chunks need zero padding
                    nc.vector.memset(wps, 0.0)
                for a in range(A1):
                    pp = min(128, R1 - 128 * a)
                    wtmp = wredp.tile([128, I], F32, tag="wred", name="wtmp")
                    nc.sync.dma_start(out=wtmp[:pp, :],
                                      in_=w1s_ap[128 * a:128 * a + pp, :])
                    nc.vector.tensor_reduce(
                        out=wps[:pp, a:a + 1], in_=wtmp[:pp, :], axis=AX,
                        op=OP.add, apply_absolute_value=True)
                for a in range(A2):
                    pp = min(128, R2 - 128 * a)
                    wtmp2 = wredp.tile([128, I], F32, tag="wred", name="wtmp2")
                    nc.sync.dma_start(out=wtmp2[:pp, :D],
                                      in_=w2s_ap[128 * a:128 * a + pp, :])
                    nc.vector.tensor_reduce(
                        out=wps[:pp, A1 + a:A1 + a + 1], in_=wtmp2[:pp, :D],
                        axis=AX, op=OP.add, apply_absolute_value=True)
                wpad = stats.tile([128, 128], F32)
                nc.vector.memset(wpad, 0.0)
                nc.vector.reduce_sum(wpad[:, 0:1], wps[:, 0:A1], axis=AX)
                nc.vector.reduce_sum(wpad[:, 1:2], wps[:, A1:A1 + A2], axis=AX)
                trw = psum.tile([128, 128], F32, tag="b", name="trw")
                nc.tensor.transpose(trw[:, :], wpad[:, :], ident[:, :])
                wred = stats.tile([8, 1], F32)
                nc.vector.memset(wred, 0.0)
                nc.vector.reduce_sum(wred[0:2, :], trw[0:2, :], axis=AX)
                nc.sync.dma_start(out=ar_in.ap()[0:8], in_=wred[:, :])
                nc.gpsimd.collective_compute(
                    "AllReduce", OP.add,
                    replica_groups=[list(range(n_cores))],
                    ins=[ar_in.ap().opt()], outs=[ar_out.ap().opt()])
                wrow = stats.tile([1, 2], F32)
                nc.sync.dma_start(out=wrow[:, :], in_=ar_out.ap()[0:2])
                mrow = stats.tile([1, 4], F32)
                nc.vector.tensor_scalar(mrow[:, 2:4], wrow[:, :], inv_cnt,
                                        1e-5, OP.mult, OP.max)
                nc.vector.reciprocal(mrow[:, 0:2], mrow[:, 2:4])
                nc.gpsimd.partition_broadcast(wsc[:, :], mrow[:, :])

            with tc.tile_pool(name="bc", bufs=2) as bc:
                # ================= Stage B: x stats + quant =================
                am1p = stats.tile([128, T], F32, tag="amp", name="am1p")
                am1n = stats.tile([128, T], F32, tag="amn", name="am1n")
                SxA = psum.tile([1, TH], F32, tag="b", name="SxA")
                SxB = psum.tile([1, TH], F32, tag="b", name="SxB")
                for k in range(KD):
                    xk = bc.tile([128, T], F32, tag="xk", name="xk")
                    nc.sync.dma_start(out=xk[:, :], in_=xT_t[k])
                    if k == 0:
                        nc.vector.tensor_copy(am1p, xk)
                        nc.vector.tensor_copy(am1n, xk)
                    else:
                        nc.vector.tensor_tensor(am1p, xk, am1p, OP.max)
                        nc.vector.tensor_tensor(am1n, xk, am1n, OP.min)
                    xsq = bc.tile([128, T], BF16, tag="xsq", name="xsq")
                    nc.scalar.activation(xsq, xk, AF.Square)
                    nc.tensor.matmul(SxA[:, :], ones_bf[:, :], xsq[:, 0:TH],
                                     start=(k == 0), stop=(k == KD - 1))
                    nc.tensor.matmul(SxB[:, :], ones_bf[:, :], xsq[:, TH:T],
                                     start=(k == 0), stop=(k == KD - 1))
                nc.vector.scalar_tensor_tensor(
                    am1n, am1n, -1.0, am1p, OP.mult, OP.max)
                Mx1 = stats.tile([128, TJ], F32)
                part_reduce_max(am1n, Mx1)
                srow1 = stats.tile([1, T], F32, tag="srow", name="srow1")
                nc.scalar.copy(srow1[:, 0:TH], SxA[:, :])
                nc.scalar.copy(srow1[:, TH:T], SxB[:, :])
                finalize_stats(Mx1, srow1, 2, sqrt_d, qs1_b, al1_b, 0)

                xqT = bc.tile([128, KD, T], BF16, tag="xqT", bufs=1,
                              name="xqT")
                for k in range(KD):
                    xk2 = bc.tile([128, T], F32, tag="xk", name="xk2")
                    nc.sync.dma_start(out=xk2[:, :], in_=xT_t[k])
                    nc.vector.tensor_tensor(xk2, xk2, qs1_b, OP.mult)
                    nc.vector.tensor_scalar(xqT[:, k, :], xk2, C_ROUND,
                                            C_ROUND, OP.add, OP.subtract)

                # ===== Stage C: layer 1 + h stats + w2 quant (interleaved) ====
                am2p = stats.tile([128, T], F32, tag="amp", name="am2p")
                am2n = stats.tile([128, T], F32, tag="amn", name="am2n")
                ShA = psum.tile([1, TH], F32, tag="b", name="ShA")
                ShB = psum.tile([1, TH], F32, tag="b", name="ShB")
                for i in range(KI):
                    w1f = bc.tile([128, KD, 128], F32, tag="w1f", name="w1f")
                    nc.sync.dma_start(out=w1f[:, :, :], in_=w1_t[i])
                    w1ff = w1f.rearrange("p k j -> p (k j)")
                    nc.scalar.activation(w1ff, w1ff, AF.Copy, scale=wsc[:, 0:1])
                    nc.vector.tensor_scalar(w1ff, w1ff, W_CLIP, -W_CLIP,
                                            OP.min, OP.max)
                    w1q = bc.tile([128, KD, 128], BF16, tag="w1q", name="w1q")
                    nc.vector.tensor_scalar(
                        w1q.rearrange("p k j -> p (k j)"), w1ff, C_ROUND,
                        C_ROUND, OP.add, OP.subtract)
                    hpsA = psum.tile([128, TH], F32, tag="b", name="hpsA")
                    hpsB = psum.tile([128, TH], F32, tag="b", name="hpsB")
                    for k in range(KD):
                        nc.tensor.matmul(hpsA[:, :], w1q[:, k, :],
                                         xqT[:, k, 0:TH],
                                         start=(k == 0), stop=(k == KD - 1))
                    for k in range(KD):
                        nc.tensor.matmul(hpsB[:, :], w1q[:, k, :],
                                         xqT[:, k, TH:T],
                                         start=(k == 0), stop=(k == KD - 1))
                    nc.vector.tensor_tensor(hpsA, hpsA, al1_b[:, 0:TH], OP.mult)
                    nc.vector.tensor_tensor(hpsB, hpsB, al1_b[:, TH:T], OP.mult)
                    h_sb = bc.tile([128, T], F32, tag="h", bufs=3, name="h_sb")
                    if gelu_mode == "gelu":
                        nc.scalar.activation(h_sb[:, 0:TH], hpsA, AF.Gelu)
                        nc.scalar.activation(h_sb[:, TH:T], hpsB, AF.Gelu)
                    else:  # sigmoid-gelu (CoreSim lacks Gelu/Erf tables)
                        gs = bc.tile([128, T], F32, tag="gsig", name="gs")
                        nc.scalar.activation(gs[:, 0:TH], hpsA, AF.Sigmoid,
                                             scale=1.702)
                        nc.scalar.activation(gs[:, TH:T], hpsB, AF.Sigmoid,
                                             scale=1.702)
                        nc.vector.tensor_tensor(h_sb[:, 0:TH], gs[:, 0:TH],
                                                hpsA, OP.mult)
                        nc.vector.tensor_tensor(h_sb[:, TH:T], gs[:, TH:T],
                                                hpsB, OP.mult)
                    nc.sync.dma_start(out=h_w[i], in_=h_sb[:, :])
                    if i == 0:
                        nc.vector.tensor_copy(am2p, h_sb)
                        nc.vector.tensor_copy(am2n, h_sb)
                    else:
                        nc.vector.tensor_tensor(am2p, h_sb, am2p, OP.max)
                        nc.vector.tensor_tensor(am2n, h_sb, am2n, OP.min)
                    hsq = bc.tile([128, T], BF16, tag="hsq", name="hsq")
                    nc.scalar.activation(hsq, h_sb, AF.Square)
                    nc.tensor.matmul(ShA[:, :], ones_bf[:, :], hsq[:, 0:TH],
                                     start=(i == 0), stop=(i == KI - 1))
                    nc.tensor.matmul(ShB[:, :], ones_bf[:, :], hsq[:, TH:T],
                                     start=(i == 0), stop=(i == KI - 1))
                    # interleaved w2 quant chunk (rows 128i .. 128i+127)
                    w2c = bc.tile([128, D], F32, tag="w2c", name="w2c")
                    nc.sync.dma_start(out=w2c[:, :], in_=w2_t[i])
                    nc.scalar.activation(w2c, w2c, AF.Copy, scale=wsc[:, 1:2])
                    nc.vector.tensor_scalar(w2c, w2c, W_CLIP, -W_CLIP,
                                            OP.min, OP.max)
                    w2qc = bc.tile([128, D], BF16, tag="w2qc", name="w2qc")
                    nc.vector.tensor_scalar(w2qc, w2c, C_ROUND, C_ROUND,
                                            OP.add, OP.subtract)
                    nc.sync.dma_start(out=w2q_w[i], in_=w2qc[:, :])

                # ---- mid stats finalize ----
                qs2_b = glob.tile([128, T], F32, tag="qsb", name="qs2_b")
                al2_b = glob.tile([128, T], F32, tag="alb", name="al2_b")
                nc.vector.scalar_tensor_tensor(
                    am2n, am2n, -1.0, am2p, OP.mult, OP.max)
                Mx2 = stats.tile([128, TJ], F32, name="Mx2")
                part_reduce_max(am2n, Mx2)
                srow2 = stats.tile([1, T], F32, tag="srow", name="srow2")
                nc.scalar.copy(srow2[:, 0:TH], ShA[:, :])
                nc.scalar.copy(srow2[:, TH:T], ShB[:, :])
                finalize_stats(Mx2, srow2, 3, sqrt_i, qs2_b, al2_b, 3)

            # ================= Stage D: quantize h, layer 2 =================
            with tc.tile_pool(name="l2", bufs=2) as l2:
                hqT = l2.tile([128, KI, T], BF16, tag="hqT", bufs=1,
                              name="hqT")
                for k2 in range(KI):
                    hk = l2.tile([128, T], F32, tag="hrd", name="hk")
                    nc.sync.dma_start(out=hk[:, :], in_=h_w[k2])
                    nc.vector.tensor_tensor(hk, hk, qs2_b, OP.mult)
                    nc.vector.tensor_scalar(hqT[:, k2, :], hk, C_ROUND,
                                            C_ROUND, OP.add, OP.subtract)
                for m in range(JD2):
                    pbank = [psum.tile([128, TH], F32, tag="b",
                                       name=f"psb{q}") for q in range(4)]
                    for kh in range(2):
                        w2qs = l2.tile([128, K2H, 256], BF16, tag="w2s",
                                       bufs=2, name="w2qs")
                        nc.sync.dma_start(
                            out=w2qs[:, :, :],
                            in_=w2q_r[m][:, ts(kh, K2H), :])
                        for kk in range(K2H):
                            k2 = kh * K2H + kk
                            first = (k2 == 0)
                            last = (k2 == KI - 1)
                            nc.tensor.matmul(pbank[0][:, :], w2qs[:, kk, 0:128],
                                             hqT[:, k2, 0:TH],
                                             start=first, stop=last)
                            nc.tensor.matmul(pbank[1][:, :], w2qs[:, kk, 0:128],
                                             hqT[:, k2, TH:T],
                                             start=first, stop=last)
                            nc.tensor.matmul(pbank[2][:, :],
                                             w2qs[:, kk, 128:256],
                                             hqT[:, k2, 0:TH],
                                             start=first, stop=last)
                            nc.tensor.matmul(pbank[3][:, :],
                                             w2qs[:, kk, 128:256],
                                             hqT[:, k2, TH:T],
                                             start=first, stop=last)
                    for jcol in range(2):
                        for half in range(2):
                            ob = l2.tile([128, TH], F32, tag="ob", bufs=2,
                                         name="ob")
                            nc.vector.tensor_tensor(
                                ob, pbank[2 * jcol + half],
                                al2_b[:, ts(half, TH)], OP.mult)
                            nc.sync.dma_start(
                                out=out_w[2 * m + jcol][:, ts(half, TH)],
                                in_=ob[:, :])

    nc.compile()  # Bacc passes: EVSEM multi-wait lowering, library loads,
    return nc     # extended-ISA codegen, nop fusion, register alloc


_NC_CACHE = {}


def _get_nc(D, I, T, n_cores):
    key = (D, I, T, n_cores)
    if key not in _NC_CACHE:
        _NC_CACHE[key] = build_bitnet(D, I, T, n_cores)
    return _NC_CACHE[key]


def make_in_maps(x, w1, w2, n_cores=N_CORES):
    """Host-side sharding/layout only (transpose + slicing, no arithmetic)."""
    xf = np.ascontiguousarray(np.asarray(x, dtype=np.float32)).reshape(
        -1, x.shape[-1])
    D = xf.shape[1]
    I = w1.shape[0]
    T = xf.shape[0] // n_cores
    w1T = np.ascontiguousarray(np.asarray(w1, dtype=np.float32).T)  # [D, I]
    w2T = np.ascontiguousarray(np.asarray(w2, dtype=np.float32).T)  # [I, D]
    in_maps = []
    for c in range(n_cores):
        xTc = np.ascontiguousarray(xf[c * T:(c + 1) * T].T)  # [D, T]
        in_maps.append({
            "xT": xTc,
            "w1T": w1T,
            "w2T": w2T,
            "w1s": np.ascontiguousarray(
                w1T[c * (D // n_cores):(c + 1) * (D // n_cores)]),
            "w2s": np.ascontiguousarray(
                w2T[c * (I // n_cores):(c + 1) * (I // n_cores)]),
        })
    return in_maps, (D, I, T)


def run_spmd(x, w1, w2, trace=False, **kwargs):
    from concourse.bass_utils import run_bass_kernel_spmd

    B, S, D = x.shape
    in_maps, (D, I, T) = make_in_maps(x, w1, w2, N_CORES)
    nc = _get_nc(D, I, T, N_CORES)
    res = run_bass_kernel_spmd(nc, in_maps, core_ids=list(range(N_CORES)),
                               trace=trace, **kwargs)
    outs = [res.results[c]["outT"].T for c in range(N_CORES)]  # each [T, D]
    out = np.concatenate(outs, axis=0).reshape(B, S, D)
    return np.ascontiguousarray(out, dtype=np.float32), res


def kernel(x, w1, w2):
    out, _ = run_spmd(x, w1, w2, trace=False)
    return out
